# revision 4
# baseline (speedup 1.0000x reference)
"""Trainium2 Bass kernel for CodeAwareContinuousEncoder (MoE-routed heads).

Computation (per sample b):
    z = clip((values - means) / max(stds, 1e-8), -5, 5)
    hidden = gelu(z * w1 + b1)                       # (H,)
    out = hidden @ W_heads[head_idx[b]] + b_heads[head_idx[b]]   # (E,)

Strategy: expert-shard the K=100 heads across 8 NeuronCores. Host-side
routing groups samples by head (index shuffling only - the shard map);
each core receives just the weights of its ~13 heads plus the
normalizer inputs of the samples routed to it, padded to a fixed
per-head capacity of 64 so all 8 cores run one identical SPMD program.
All arithmetic runs on-device.

v4 (fp16): weights, hidden, z-broadcast and outputs are fp16 (PSUM
accumulation stays fp32). vs the fp32 v3 this quarters the PE matmul
time (no LOW/HIGH limb passes) and halves the weight-DMA bytes
(1.7 MB/core), which is the critical path in the memory-bound regime.

Per-core dataflow:
  - z on DVE in a (128, M) fp32 layout, cast to fp16, flattened to a
    (1, N) row by one SBUF->SBUF HWDGE DMA on the scalar ring
  - z broadcast across partitions by a rank-1 PE matmul ones^T x z into
    PSUM (fp16 moving, fp32 accumulate); ACT Gelu chunks read it with
    per-partition scale=w1 / bias=b1 APs and emit fp16 hidden
  - two segments share one (128, E) PSUM tile: a rank-2 "split-row"
    bias matmul mask(2,128)^T x [bg_even; bg_odd](2,E) initializes the
    tile (start=True) with each half's head bias - these run before the
    weights arrive and double as PE HAM warm-up - then 4 fp16 weight
    matmuls accumulate via column-group packing (tile_position 0/64)
  - PSUM->SBUF drain casts to fp16 (DVE/ACT alternating), one output
    DMA per pair on the scalar ring
  - weight pair DMAs stream back-to-back on the sync HWDGE ring only
    (2 KB/partition descriptors), small inputs lead on the scalar ring
"""

import numpy as np
from contextlib import ExitStack

import concourse.bass as bass
import concourse.tile as tile
from concourse import bacc, mybir
from concourse.bass_utils import run_bass_kernel_spmd
from concourse.tile_rust import add_dep_helper

B, H, E, K = 4096, 256, 256, 100
NCORES = 8
P = 128
CAP = 64
F32 = mybir.dt.float32
F16 = mybir.dt.float16

TRACE = False
LAST_RESULT = None

_build_cache = {}


def _build(S, MCOLS):
    """SPMD per-core program. S segments of 64 slots; MCOLS z columns."""
    nc = bacc.Bacc("TRN2", target_bir_lowering=False, debug=False)
    M = MCOLS
    N = P * M
    PAIRS = (S + 1) // 2
    NBIAS = PAIRS * E

    # [p, 0:M]=v [M:2M]=m [2M:3M]=s [3M:3M+2]=b1col [3M+2:3M+4]=w1col
    vmsb = nc.dram_tensor("vmsb", [P, 3 * M + 4], F32, kind="ExternalInput").ap()
    # row j: [i*E:(i+1)*E] = b_heads[head of segment 2i+j]; [NBIAS:NBIAS+P] = mask row j
    bgm = nc.dram_tensor("bgm", [2, NBIAS + P], F16, kind="ExternalInput").ap()
    # wg[p, s*2E + c*E + e] = W_heads[head_s, c*128 + p, e]
    wg = nc.dram_tensor("wg", [P, S * 2 * E], F16, kind="ExternalInput").ap()
    # pair layout: rows [0:64] = seg 2i, rows [64:128] = seg 2i+1
    y = nc.dram_tensor("y", [PAIRS, P, E], F16, kind="ExternalOutput").ap()

    with tile.TileContext(nc) as tc, ExitStack() as ctx:
        const_pool = ctx.enter_context(tc.tile_pool(name="const", bufs=1))
        zpool = ctx.enter_context(tc.tile_pool(name="z", bufs=1))
        hpool = ctx.enter_context(tc.tile_pool(name="hidden", bufs=1))
        wpool = ctx.enter_context(tc.tile_pool(name="w", bufs=PAIRS))
        opool = ctx.enter_context(tc.tile_pool(name="osb", bufs=4))
        pp = ctx.enter_context(tc.tile_pool(name="psum", bufs=6, space="PSUM"))
        php = ctx.enter_context(tc.tile_pool(name="ph", bufs=1, space="PSUM"))

        # --- ACT Gelu table preload first: the table DMA (queue 14) runs
        # while the input DMAs stream, and the scalar engine is free again
        # before the first real gelu.
        gscr = const_pool.tile([P, 1], F32)
        nc.vector.memset(gscr[:], 0.5)
        gscr2 = const_pool.tile([P, 1], F32)
        nc.scalar.activation(gscr2[:], gscr[:], mybir.ActivationFunctionType.Gelu)

        # --- small inputs on the scalar ring; weight pairs stream on sync
        t_vmsb = const_pool.tile([P, 3 * M + 4], F32)
        nc.scalar.dma_start(t_vmsb[:], vmsb)
        t_bgm = const_pool.tile([2, NBIAS + P], F16)
        nc.scalar.dma_start(t_bgm[:], bgm)

        wts = []
        for i in range(PAIRS):
            lo = i * 2 * 2 * E
            sz = min(2 * 2 * E, S * 2 * E - lo)
            wt = wpool.tile([P, 2 * 2 * E], F16, tag="wt")
            nc.sync.dma_start(wt[:, 0:sz], wg[:, lo : lo + sz])
            wts.append(wt)

        ones_row = const_pool.tile([1, P], F16)
        nc.vector.memset(ones_row[:], 1.0)

        # --- bias matmuls: initialize each pair's PSUM tile with the two
        # head biases in split rows; early (only needs bgm) = PE warm-up.
        pos = []
        bias_mms = []
        msk = t_bgm[:, NBIAS : NBIAS + P]
        for i in range(PAIRS):
            po = pp.tile([P, E], F32, tag="po")
            pos.append(po)
            mm = nc.tensor.matmul(
                po[:],
                msk,
                t_bgm[:, i * E : (i + 1) * E],
                start=True,
                stop=False,
                skip_group_check=True,
            )
            bias_mms.append(mm)
            if i >= PAIRS - 2:
                break  # last pair psum allocated later (bank budget)

        # --- z = clip((v - m) * recip(max(s, 1e-8)), -5, 5) on (128, M)
        z2d = zpool.tile([P, M], F32)
        tmp = zpool.tile([P, M], F32)
        nc.vector.tensor_sub(z2d[:], t_vmsb[:, 0:M], t_vmsb[:, M : 2 * M])
        nc.vector.tensor_scalar_max(tmp[:], t_vmsb[:, 2 * M : 3 * M], 1e-8)
        rec = zpool.tile([P, M], F32)
        nc.vector.reciprocal(rec[:], tmp[:])
        nc.vector.tensor_mul(z2d[:], z2d[:], rec[:])
        nc.vector.tensor_scalar(
            z2d[:], z2d[:], 5.0, -5.0, mybir.AluOpType.min, mybir.AluOpType.max
        )
        z2dh = zpool.tile([P, M], F16)
        nc.vector.tensor_copy(z2dh[:], z2d[:])

        # --- flatten slot-major via SBUF->SBUF DMA on the scalar HWDGE ring
        zrow = zpool.tile([1, N], F16)
        nc.scalar.dma_start(zrow[:].rearrange("a (p m) -> a p m", p=P), z2dh[:])
        ph = php.tile([P, N], F32)
        half = (N // 2 + 127) // 128 * 128  # multiple of 128, <= 512
        assert half <= 512
        zb_bounds = [(0, half), (half, N)]
        for lo, hi in zb_bounds:
            nc.tensor.matmul(
                ph[:, lo:hi],
                ones_row[:],
                zrow[:, lo:hi],
                start=True,
                stop=True,
            )

        # --- hidden chunks: h[c2][p, i] = gelu(z_i * w1[c2*128+p] + b1[..])
        # 256-col chunks, c2-interleaved, so each pair's matmuls only wait
        # on the gelu chunk covering its own 128 columns
        hid = []
        for c2 in range(2):
            h = hpool.tile([P, N], F16, tag=f"h{c2}", name=f"h{c2}")
            hid.append(h)
        gelu_bounds = []
        for lo, hi in zb_bounds:
            for b in range(lo, hi, 256):
                gelu_bounds.append((b, min(b + 256, hi)))
        for lo, hi in gelu_bounds:
            for c2 in range(2):
                nc.scalar.activation(
                    hid[c2][:, lo:hi],
                    ph[:, lo:hi],
                    mybir.ActivationFunctionType.Gelu,
                    scale=t_vmsb[:, 3 * M + 2 + c2 : 3 * M + 3 + c2],
                    bias=t_vmsb[:, 3 * M + c2 : 3 * M + 1 + c2],
                )

        # --- segment pair GEMMs, column-group packed
        for i in range(PAIRS):
            if i >= len(pos):
                po = pp.tile([P, E], F32, tag="po")
                pos.append(po)
                bias_mms.append(
                    nc.tensor.matmul(
                        po[:],
                        msk,
                        t_bgm[:, i * E : (i + 1) * E],
                        start=True,
                        stop=False,
                        skip_group_check=True,
                    )
                )
            wt = wts[i]
            po = pos[i]
            segs = [2 * i] + ([2 * i + 1] if (2 * i + 1) < S else [])
            last_mm = None
            for c2 in range(2):
                for j, s in enumerate(segs):
                    colbase = 64 * j
                    last_mm = nc.tensor.matmul(
                        po[colbase : colbase + CAP, :],
                        hid[c2][:, s * CAP : (s + 1) * CAP],
                        wt[:, (2 * j + c2) * E : (2 * j + c2 + 1) * E],
                        start=False,
                        stop=(c2 == 1),
                        tile_position=(0, colbase),
                        skip_group_check=True,
                    )
            osb = opool.tile([P, E], F16, tag="osb")
            if i % 2 == 0:
                cp = nc.vector.tensor_copy(osb[:], po[:])
            else:
                cp = nc.scalar.copy(osb[:], po[:])
            # copy reads the whole tile; deps already cover all matmuls,
            # but order explicitly after the final matmul for bank safety
            add_dep_helper(cp.ins, last_mm.ins, True, "psum drain order")
            nc.scalar.dma_start(y[i], osb[:])
    nc.compile()
    return nc


def kernel(values, means, stds, head_idx, w1, b1, W_heads, b_heads):
    global LAST_RESULT
    values = np.ascontiguousarray(values, dtype=np.float32)
    means = np.ascontiguousarray(means, dtype=np.float32)
    stds = np.ascontiguousarray(stds, dtype=np.float32)
    head_idx = np.ascontiguousarray(head_idx, dtype=np.int32)
    w1 = np.ascontiguousarray(w1, dtype=np.float32)
    b1 = np.ascontiguousarray(b1, dtype=np.float32)
    W_heads = np.ascontiguousarray(W_heads, dtype=np.float32)
    b_heads = np.ascontiguousarray(b_heads, dtype=np.float32)
    nb = values.shape[0]

    # ---- host routing: group sample indices by head, chunk to <=64 ----
    order = np.argsort(head_idx, kind="stable")
    counts = np.bincount(head_idx, minlength=K)
    bounds = np.concatenate([[0], np.cumsum(counts)])
    segments = []  # (head, idx_array)
    for k in range(K):
        idx = order[bounds[k] : bounds[k + 1]]
        for lo in range(0, len(idx), CAP):
            segments.append((k, idx[lo : lo + CAP]))
    S = -(-len(segments) // NCORES)
    while len(segments) < S * NCORES:
        segments.append((0, np.empty(0, dtype=np.int64)))
    MCOLS = -(-(S * CAP) // P)
    N = P * MCOLS
    PAIRS = (S + 1) // 2
    NBIAS = PAIRS * E

    key = (S, MCOLS)
    if key not in _build_cache:
        _build_cache[key] = _build(S, MCOLS)
    nc = _build_cache[key]

    b1col = b1.reshape(2, P).T  # (128, 2)
    w1col = w1.reshape(2, P).T  # (128, 2)
    # (K, 128, 2, E): [k, p, c, e] = W_heads[k, c*128+p, e]
    W_chunked = np.ascontiguousarray(
        W_heads.reshape(K, 2, P, E).transpose(0, 2, 1, 3).astype(np.float16)
    )

    in_maps = []
    core_segs = []
    for c in range(NCORES):
        segs = segments[c * S : (c + 1) * S]
        core_segs.append(segs)
        v_slot = np.zeros(N, np.float32)
        m_slot = np.zeros(N, np.float32)
        s_slot = np.ones(N, np.float32)
        for si, (k, idx) in enumerate(segs):
            n = len(idx)
            sl = slice(si * CAP, si * CAP + n)
            v_slot[sl] = values[idx]
            m_slot[sl] = means[idx]
            s_slot[sl] = stds[idx]
        vmsb = np.empty((P, 3 * MCOLS + 4), np.float32)
        vmsb[:, 0:MCOLS] = v_slot.reshape(P, MCOLS)
        vmsb[:, MCOLS : 2 * MCOLS] = m_slot.reshape(P, MCOLS)
        vmsb[:, 2 * MCOLS : 3 * MCOLS] = s_slot.reshape(P, MCOLS)
        vmsb[:, 3 * MCOLS : 3 * MCOLS + 2] = b1col
        vmsb[:, 3 * MCOLS + 2 : 3 * MCOLS + 4] = w1col
        heads = np.array([k for k, _ in segs], np.int64)
        bgm = np.zeros((2, NBIAS + P), np.float16)
        bg = b_heads[heads].astype(np.float16)  # (S, E)
        bgm[0, : (len(segs) + 1) // 2 * E] = bg[0::2].reshape(-1)
        bgm[1, : len(segs) // 2 * E] = bg[1::2].reshape(-1)
        bgm[0, NBIAS : NBIAS + CAP] = 1.0
        bgm[1, NBIAS + CAP : NBIAS + P] = 1.0
        # (128, S*2*E) segment-major, per-partition contiguous
        wgc = np.ascontiguousarray(
            W_chunked[heads].transpose(1, 0, 2, 3).reshape(P, S * 2 * E)
        )
        in_maps.append({"vmsb": vmsb, "bgm": bgm, "wg": wgc})

    # Walrus emits a NEFF epilogue that re-zeros every semaphore in
    # [3, max-sem-num) one instruction at a time (~7 us measured). Our
    # TileContext already range-clears the bass-managed sems it used, so
    # shrink the walrus clear range to its own semaphore space.
    from concourse.compiler_utils import temporarily_append_compiler_flags

    with temporarily_append_compiler_flags(
        ["--internal-backend-options=--max-sem-num=150"]
    ):
        res = run_bass_kernel_spmd(nc, in_maps, list(range(NCORES)), trace=TRACE)
    LAST_RESULT = res

    out = np.empty((nb, E), np.float32)
    for c in range(NCORES):
        yc = np.asarray(res.results[c]["y"], dtype=np.float32)  # (PAIRS, 128, E)
        for si, (k, idx) in enumerate(core_segs[c]):
            n = len(idx)
            if n:
                out[idx] = yc[si // 2, CAP * (si % 2) : CAP * (si % 2) + n, :]
    return out


# revision 6
# speedup vs baseline: 1.0204x; 1.0204x over previous
"""Trainium2 Bass kernel for CodeAwareContinuousEncoder (MoE-routed heads).

Computation (per sample b):
    z = clip((values - means) / max(stds, 1e-8), -5, 5)
    hidden = gelu(z * w1 + b1)                       # (H,)
    out = hidden @ W_heads[head_idx[b]] + b_heads[head_idx[b]]   # (E,)

Strategy: expert-shard the K=100 heads across 8 NeuronCores. Host-side
routing groups samples by head (index shuffling only - the shard map);
each core receives just the weights of its ~13 heads plus the
normalizer inputs of the samples routed to it, padded to a fixed
per-head capacity of 64 so all 8 cores run one identical SPMD program.
All arithmetic runs on-device.

v4 (fp16): weights, hidden, z-broadcast and outputs are fp16 (PSUM
accumulation stays fp32). vs the fp32 v3 this quarters the PE matmul
time (no LOW/HIGH limb passes) and halves the weight-DMA bytes
(1.7 MB/core), which is the critical path in the memory-bound regime.

Per-core dataflow:
  - z on DVE in a (128, M) fp32 layout, cast to fp16, flattened to a
    (1, N) row by one SBUF->SBUF HWDGE DMA on the scalar ring
  - z broadcast across partitions by a rank-1 PE matmul ones^T x z into
    PSUM (fp16 moving, fp32 accumulate); ACT Gelu chunks read it with
    per-partition scale=w1 / bias=b1 APs and emit fp16 hidden
  - two segments share one (128, E) PSUM tile: a rank-2 "split-row"
    bias matmul mask(2,128)^T x [bg_even; bg_odd](2,E) initializes the
    tile (start=True) with each half's head bias - these run before the
    weights arrive and double as PE HAM warm-up - then 4 fp16 weight
    matmuls accumulate via column-group packing (tile_position 0/64)
  - PSUM->SBUF drain casts to fp16 (DVE/ACT alternating), one output
    DMA per pair on the scalar ring
  - weight pair DMAs stream back-to-back on the sync HWDGE ring only
    (2 KB/partition descriptors), small inputs lead on the scalar ring
"""

import numpy as np
from contextlib import ExitStack

import concourse.bass as bass
import concourse.tile as tile
from concourse import bacc, mybir
from concourse.bass_utils import run_bass_kernel_spmd
from concourse.tile_rust import add_dep_helper

B, H, E, K = 4096, 256, 256, 100
NCORES = 8
P = 128
CAP = 64
F32 = mybir.dt.float32
F16 = mybir.dt.float16

TRACE = False
LAST_RESULT = None

_build_cache = {}

# Walrus emits a NEFF epilogue that re-zeros every semaphore in
# [3, max-sem-num) one instruction at a time (~7 us measured, ~30% of
# kernel wall time). Our TileContext already range-clears the
# bass-managed sems it used, so shrink the walrus clear range to its
# own semaphore space by passing --max-sem-num to walrus_driver.
import concourse.bass_utils as _bass_utils_mod

if not getattr(_bass_utils_mod, "_walrus_max_sem_patch", False):
    _orig_get_walrus_args = _bass_utils_mod.get_walrus_args

    def _get_walrus_args_max_sem(arch, tmpdir, *, dve_root=None):
        return _orig_get_walrus_args(arch, tmpdir, dve_root=dve_root) + [
            "--max-sem-num=150"
        ]

    _bass_utils_mod.get_walrus_args = _get_walrus_args_max_sem
    _bass_utils_mod._walrus_max_sem_patch = True


def _build(S, MCOLS):
    """SPMD per-core program. S segments of 64 slots; MCOLS z columns."""
    nc = bacc.Bacc("TRN2", target_bir_lowering=False, debug=False)
    M = MCOLS
    N = P * M
    PAIRS = (S + 1) // 2
    NBIAS = PAIRS * E

    # [p, 0:M]=v [M:2M]=m [2M:3M]=s [3M:3M+2]=b1col [3M+2:3M+4]=w1col
    vmsb = nc.dram_tensor("vmsb", [P, 3 * M + 4], F32, kind="ExternalInput").ap()
    # row j: [i*E:(i+1)*E] = b_heads[head of segment 2i+j]; [NBIAS:NBIAS+P] = mask row j
    bgm = nc.dram_tensor("bgm", [2, NBIAS + P], F16, kind="ExternalInput").ap()
    # wg[p, s*2E + c*E + e] = W_heads[head_s, c*128 + p, e]
    wg = nc.dram_tensor("wg", [P, S * 2 * E], F16, kind="ExternalInput").ap()
    # pair layout: rows [0:64] = seg 2i, rows [64:128] = seg 2i+1
    y = nc.dram_tensor("y", [PAIRS, P, E], F16, kind="ExternalOutput").ap()

    with tile.TileContext(nc) as tc, ExitStack() as ctx:
        const_pool = ctx.enter_context(tc.tile_pool(name="const", bufs=1))
        zpool = ctx.enter_context(tc.tile_pool(name="z", bufs=1))
        hpool = ctx.enter_context(tc.tile_pool(name="hidden", bufs=1))
        wpool = ctx.enter_context(tc.tile_pool(name="w", bufs=PAIRS))
        opool = ctx.enter_context(tc.tile_pool(name="osb", bufs=4))
        pp = ctx.enter_context(tc.tile_pool(name="psum", bufs=6, space="PSUM"))
        php = ctx.enter_context(tc.tile_pool(name="ph", bufs=1, space="PSUM"))

        # --- ACT Gelu table preload first: the table DMA (queue 14) runs
        # while the input DMAs stream, and the scalar engine is free again
        # before the first real gelu.
        gscr = const_pool.tile([P, 1], F32)
        nc.vector.memset(gscr[:], 0.5)
        gscr2 = const_pool.tile([P, 1], F32)
        nc.scalar.activation(gscr2[:], gscr[:], mybir.ActivationFunctionType.Gelu)

        # --- small inputs on the scalar ring; weight pairs stream on sync
        t_vmsb = const_pool.tile([P, 3 * M + 4], F32)
        nc.scalar.dma_start(t_vmsb[:], vmsb)
        t_bgm = const_pool.tile([2, NBIAS + P], F16)
        nc.scalar.dma_start(t_bgm[:], bgm)

        wts = []
        for i in range(PAIRS):
            lo = i * 2 * 2 * E
            sz = min(2 * 2 * E, S * 2 * E - lo)
            wt = wpool.tile([P, 2 * 2 * E], F16, tag="wt")
            nc.sync.dma_start(wt[:, 0:sz], wg[:, lo : lo + sz])
            wts.append(wt)

        ones_row = const_pool.tile([1, P], F16)
        nc.vector.memset(ones_row[:], 1.0)

        # --- bias matmuls: initialize each pair's PSUM tile with the two
        # head biases in split rows; early (only needs bgm) = PE warm-up.
        pos = []
        bias_mms = []
        msk = t_bgm[:, NBIAS : NBIAS + P]
        for i in range(PAIRS):
            po = pp.tile([P, E], F32, tag="po")
            pos.append(po)
            mm = nc.tensor.matmul(
                po[:],
                msk,
                t_bgm[:, i * E : (i + 1) * E],
                start=True,
                stop=False,
                skip_group_check=True,
            )
            bias_mms.append(mm)
            if i >= PAIRS - 2:
                break  # last pair psum allocated later (bank budget)

        # --- z = clip((v - m) * recip(max(s, 1e-8)), -5, 5) on (128, M)
        z2d = zpool.tile([P, M], F32)
        tmp = zpool.tile([P, M], F32)
        nc.vector.tensor_sub(z2d[:], t_vmsb[:, 0:M], t_vmsb[:, M : 2 * M])
        nc.vector.tensor_scalar_max(tmp[:], t_vmsb[:, 2 * M : 3 * M], 1e-8)
        rec = zpool.tile([P, M], F32)
        nc.vector.reciprocal(rec[:], tmp[:])
        nc.vector.tensor_mul(z2d[:], z2d[:], rec[:])
        nc.vector.tensor_scalar(
            z2d[:], z2d[:], 5.0, -5.0, mybir.AluOpType.min, mybir.AluOpType.max
        )
        z2dh = zpool.tile([P, M], F16)
        nc.vector.tensor_copy(z2dh[:], z2d[:])

        # --- flatten slot-major via SBUF->SBUF DMA on the scalar HWDGE ring
        zrow = zpool.tile([1, N], F16)
        nc.scalar.dma_start(zrow[:].rearrange("a (p m) -> a p m", p=P), z2dh[:])
        ph = php.tile([P, N], F32)
        half = (N // 2 + 127) // 128 * 128  # multiple of 128, <= 512
        assert half <= 512
        zb_bounds = [(0, half), (half, N)]
        for lo, hi in zb_bounds:
            nc.tensor.matmul(
                ph[:, lo:hi],
                ones_row[:],
                zrow[:, lo:hi],
                start=True,
                stop=True,
            )

        # --- hidden chunks: h[c2][p, i] = gelu(z_i * w1[c2*128+p] + b1[..])
        # 256-col chunks, c2-interleaved, so each pair's matmuls only wait
        # on the gelu chunk covering its own 128 columns
        hid = []
        for c2 in range(2):
            h = hpool.tile([P, N], F16, tag=f"h{c2}", name=f"h{c2}")
            hid.append(h)
        gelu_bounds = []
        for lo, hi in zb_bounds:
            for b in range(lo, hi, 256):
                gelu_bounds.append((b, min(b + 256, hi)))
        for lo, hi in gelu_bounds:
            for c2 in range(2):
                nc.scalar.activation(
                    hid[c2][:, lo:hi],
                    ph[:, lo:hi],
                    mybir.ActivationFunctionType.Gelu,
                    scale=t_vmsb[:, 3 * M + 2 + c2 : 3 * M + 3 + c2],
                    bias=t_vmsb[:, 3 * M + c2 : 3 * M + 1 + c2],
                )

        # --- segment pair GEMMs, column-group packed
        for i in range(PAIRS):
            if i >= len(pos):
                po = pp.tile([P, E], F32, tag="po")
                pos.append(po)
                bias_mms.append(
                    nc.tensor.matmul(
                        po[:],
                        msk,
                        t_bgm[:, i * E : (i + 1) * E],
                        start=True,
                        stop=False,
                        skip_group_check=True,
                    )
                )
            wt = wts[i]
            po = pos[i]
            segs = [2 * i] + ([2 * i + 1] if (2 * i + 1) < S else [])
            last_mm = None
            for c2 in range(2):
                for j, s in enumerate(segs):
                    colbase = 64 * j
                    last_mm = nc.tensor.matmul(
                        po[colbase : colbase + CAP, :],
                        hid[c2][:, s * CAP : (s + 1) * CAP],
                        wt[:, (2 * j + c2) * E : (2 * j + c2 + 1) * E],
                        start=False,
                        stop=(c2 == 1),
                        tile_position=(0, colbase),
                        skip_group_check=True,
                    )
            osb = opool.tile([P, E], F16, tag="osb")
            if i % 2 == 0:
                cp = nc.vector.tensor_copy(osb[:], po[:])
            else:
                cp = nc.scalar.copy(osb[:], po[:])
            # copy reads the whole tile; deps already cover all matmuls,
            # but order explicitly after the final matmul for bank safety
            add_dep_helper(cp.ins, last_mm.ins, True, "psum drain order")
            nc.scalar.dma_start(y[i], osb[:])
    nc.compile()
    return nc


def kernel(values, means, stds, head_idx, w1, b1, W_heads, b_heads):
    global LAST_RESULT
    values = np.ascontiguousarray(values, dtype=np.float32)
    means = np.ascontiguousarray(means, dtype=np.float32)
    stds = np.ascontiguousarray(stds, dtype=np.float32)
    head_idx = np.ascontiguousarray(head_idx, dtype=np.int32)
    w1 = np.ascontiguousarray(w1, dtype=np.float32)
    b1 = np.ascontiguousarray(b1, dtype=np.float32)
    W_heads = np.ascontiguousarray(W_heads, dtype=np.float32)
    b_heads = np.ascontiguousarray(b_heads, dtype=np.float32)
    nb = values.shape[0]

    # ---- host routing: group sample indices by head, chunk to <=64 ----
    order = np.argsort(head_idx, kind="stable")
    counts = np.bincount(head_idx, minlength=K)
    bounds = np.concatenate([[0], np.cumsum(counts)])
    segments = []  # (head, idx_array)
    for k in range(K):
        idx = order[bounds[k] : bounds[k + 1]]
        for lo in range(0, len(idx), CAP):
            segments.append((k, idx[lo : lo + CAP]))
    S = -(-len(segments) // NCORES)
    while len(segments) < S * NCORES:
        segments.append((0, np.empty(0, dtype=np.int64)))
    MCOLS = -(-(S * CAP) // P)
    N = P * MCOLS
    PAIRS = (S + 1) // 2
    NBIAS = PAIRS * E

    key = (S, MCOLS)
    if key not in _build_cache:
        _build_cache[key] = _build(S, MCOLS)
    nc = _build_cache[key]

    b1col = b1.reshape(2, P).T  # (128, 2)
    w1col = w1.reshape(2, P).T  # (128, 2)
    # (K, 128, 2, E): [k, p, c, e] = W_heads[k, c*128+p, e]
    W_chunked = np.ascontiguousarray(
        W_heads.reshape(K, 2, P, E).transpose(0, 2, 1, 3).astype(np.float16)
    )

    in_maps = []
    core_segs = []
    for c in range(NCORES):
        segs = segments[c * S : (c + 1) * S]
        core_segs.append(segs)
        v_slot = np.zeros(N, np.float32)
        m_slot = np.zeros(N, np.float32)
        s_slot = np.ones(N, np.float32)
        for si, (k, idx) in enumerate(segs):
            n = len(idx)
            sl = slice(si * CAP, si * CAP + n)
            v_slot[sl] = values[idx]
            m_slot[sl] = means[idx]
            s_slot[sl] = stds[idx]
        vmsb = np.empty((P, 3 * MCOLS + 4), np.float32)
        vmsb[:, 0:MCOLS] = v_slot.reshape(P, MCOLS)
        vmsb[:, MCOLS : 2 * MCOLS] = m_slot.reshape(P, MCOLS)
        vmsb[:, 2 * MCOLS : 3 * MCOLS] = s_slot.reshape(P, MCOLS)
        vmsb[:, 3 * MCOLS : 3 * MCOLS + 2] = b1col
        vmsb[:, 3 * MCOLS + 2 : 3 * MCOLS + 4] = w1col
        heads = np.array([k for k, _ in segs], np.int64)
        bgm = np.zeros((2, NBIAS + P), np.float16)
        bg = b_heads[heads].astype(np.float16)  # (S, E)
        bgm[0, : (len(segs) + 1) // 2 * E] = bg[0::2].reshape(-1)
        bgm[1, : len(segs) // 2 * E] = bg[1::2].reshape(-1)
        bgm[0, NBIAS : NBIAS + CAP] = 1.0
        bgm[1, NBIAS + CAP : NBIAS + P] = 1.0
        # (128, S*2*E) segment-major, per-partition contiguous
        wgc = np.ascontiguousarray(
            W_chunked[heads].transpose(1, 0, 2, 3).reshape(P, S * 2 * E)
        )
        in_maps.append({"vmsb": vmsb, "bgm": bgm, "wg": wgc})

    res = run_bass_kernel_spmd(nc, in_maps, list(range(NCORES)), trace=TRACE)
    LAST_RESULT = res

    out = np.empty((nb, E), np.float32)
    for c in range(NCORES):
        yc = np.asarray(res.results[c]["y"], dtype=np.float32)  # (PAIRS, 128, E)
        for si, (k, idx) in enumerate(core_segs[c]):
            n = len(idx)
            if n:
                out[idx] = yc[si // 2, CAP * (si % 2) : CAP * (si % 2) + n, :]
    return out


# revision 13
# speedup vs baseline: 1.0705x; 1.0491x over previous
"""Trainium2 Bass kernel for CodeAwareContinuousEncoder (MoE-routed heads).

Computation (per sample b):
    z = clip((values - means) / max(stds, 1e-8), -5, 5)
    hidden = gelu(z * w1 + b1)                       # (H,)
    out = hidden @ W_heads[head_idx[b]] + b_heads[head_idx[b]]   # (E,)

Strategy: expert-shard the K=100 heads across 8 NeuronCores. Host-side
routing groups samples by head (index shuffling only - the shard map);
each core receives just the weights of its ~13 heads plus the
normalizer inputs of the samples routed to it, padded to a fixed
per-head capacity of 64 so all 8 cores run one identical SPMD program.
All arithmetic runs on-device.

v4 (fp16): weights, hidden, z-broadcast and outputs are fp16 (PSUM
accumulation stays fp32). vs the fp32 v3 this quarters the PE matmul
time (no LOW/HIGH limb passes) and halves the weight-DMA bytes
(1.7 MB/core), which is the critical path in the memory-bound regime.

Per-core dataflow:
  - z on DVE in a (128, M) fp32 layout, cast to fp16, flattened to a
    (1, N) row by one SBUF->SBUF HWDGE DMA on the scalar ring
  - z broadcast across partitions by a rank-1 PE matmul ones^T x z into
    PSUM (fp16 moving, fp32 accumulate); ACT Gelu chunks read it with
    per-partition scale=w1 / bias=b1 APs and emit fp16 hidden
  - two segments share one (128, E) PSUM tile: a rank-2 "split-row"
    bias matmul mask(2,128)^T x [bg_even; bg_odd](2,E) initializes the
    tile (start=True) with each half's head bias - these run before the
    weights arrive and double as PE HAM warm-up - then 4 fp16 weight
    matmuls accumulate via column-group packing (tile_position 0/64)
  - PSUM->SBUF drain casts to fp16 (DVE/ACT alternating), one output
    DMA per pair on the scalar ring
  - weight pair DMAs stream back-to-back on the sync HWDGE ring only
    (2 KB/partition descriptors), small inputs lead on the scalar ring
"""

import numpy as np
from contextlib import ExitStack

import concourse.bass as bass
import concourse.tile as tile
from concourse import bacc, mybir
from concourse.bass_utils import run_bass_kernel_spmd
from concourse.tile_rust import add_dep_helper

B, H, E, K = 4096, 256, 256, 100
NCORES = 8
P = 128
CAP = 64
F32 = mybir.dt.float32
F16 = mybir.dt.float16

TRACE = False
LAST_RESULT = None

_build_cache = {}




def _build(S, MCOLS):
    """SPMD per-core program. S segments of 64 slots; MCOLS z columns."""
    nc = bacc.Bacc("TRN2", target_bir_lowering=False, debug=False)
    M = MCOLS
    N = P * M
    PAIRS = (S + 1) // 2
    NBIAS = PAIRS * E

    # [p, 0:M]=v [M:2M]=m [2M:3M]=s [3M:3M+2]=b1col [3M+2:3M+4]=w1col
    vmsb = nc.dram_tensor("vmsb", [P, 3 * M + 4], F32, kind="ExternalInput").ap()
    # row j: [i*E:(i+1)*E] = b_heads[head of segment 2i+j]; [NBIAS:NBIAS+P] = mask row j
    bgm = nc.dram_tensor("bgm", [2, NBIAS + P], F16, kind="ExternalInput").ap()
    # wg[p, s*2E + c*E + e] = W_heads[head_s, c*128 + p, e]
    wg = nc.dram_tensor("wg", [P, S * 2 * E], F16, kind="ExternalInput").ap()
    # y[p, i*E + e]: pair i, row p (rows [0:64] = seg 2i, [64:128] = seg 2i+1)
    y = nc.dram_tensor("y", [P, PAIRS * E], F16, kind="ExternalOutput").ap()

    with tile.TileContext(nc) as tc, ExitStack() as ctx:
        const_pool = ctx.enter_context(tc.tile_pool(name="const", bufs=1))
        zpool = ctx.enter_context(tc.tile_pool(name="z", bufs=1))
        hpool = ctx.enter_context(tc.tile_pool(name="hidden", bufs=1))
        wpool = ctx.enter_context(tc.tile_pool(name="w", bufs=PAIRS))
        opool = ctx.enter_context(tc.tile_pool(name="osb", bufs=1))
        pp = ctx.enter_context(tc.tile_pool(name="psum", bufs=6, space="PSUM"))
        php = ctx.enter_context(tc.tile_pool(name="ph", bufs=1, space="PSUM"))

        # --- ACT Gelu table preload first: the table DMA (queue 14) runs
        # while the input DMAs stream, and the scalar engine is free again
        # before the first real gelu. All PSUM drains go to DVE so Gelu is
        # the only ACT function used - exactly one table load.
        gscr = const_pool.tile([P, 1], F32)
        nc.vector.memset(gscr[:], 0.5)
        gscr2 = const_pool.tile([P, 1], F16)
        nc.scalar.activation(gscr2[:], gscr[:], mybir.ActivationFunctionType.Gelu)

        # --- small inputs: vmsb leads the sync ring (z path latency), bgm
        # on the scalar ring; the 7 weight pair DMAs stream on sync behind
        # vmsb. Each dma_start costs ~0.6-0.7us of issuing-engine time
        # (~5ns x 128 partition descriptors), so the rings are balanced by
        # instruction count, not bytes.
        t_vmsb = const_pool.tile([P, 3 * M + 4], F32)
        nc.sync.dma_start(t_vmsb[:], vmsb)
        t_bgm = const_pool.tile([2, NBIAS + P], F16)
        nc.scalar.dma_start(t_bgm[:], bgm)

        wts = []
        for i in range(PAIRS):
            lo = i * 2 * 2 * E
            sz = min(2 * 2 * E, S * 2 * E - lo)
            wt = wpool.tile([P, 2 * 2 * E], F16, tag="wt")
            nc.sync.dma_start(wt[:, 0:sz], wg[:, lo : lo + sz])
            wts.append(wt)

        ones_row = const_pool.tile([1, P], F16)
        nc.vector.memset(ones_row[:], 1.0)

        # --- bias matmuls: initialize each pair's PSUM tile with the two
        # head biases in split rows; early (only needs bgm) = PE warm-up.
        pos = []
        bias_mms = []
        msk = t_bgm[:, NBIAS : NBIAS + P]
        for i in range(PAIRS):
            po = pp.tile([P, E], F32, tag="po")
            pos.append(po)
            mm = nc.tensor.matmul(
                po[:],
                msk,
                t_bgm[:, i * E : (i + 1) * E],
                start=True,
                stop=False,
                skip_group_check=True,
            )
            bias_mms.append(mm)
            if i >= PAIRS - 2:
                break  # last pair psum allocated later (bank budget)

        # --- z = clip((v - m) * recip(max(s, 1e-8)), -5, 5) on (128, M)
        z2d = zpool.tile([P, M], F32)
        tmp = zpool.tile([P, M], F32)
        nc.vector.tensor_sub(z2d[:], t_vmsb[:, 0:M], t_vmsb[:, M : 2 * M])
        nc.vector.tensor_scalar_max(tmp[:], t_vmsb[:, 2 * M : 3 * M], 1e-8)
        rec = zpool.tile([P, M], F32)
        nc.vector.reciprocal(rec[:], tmp[:])
        nc.vector.tensor_mul(z2d[:], z2d[:], rec[:])
        nc.vector.tensor_scalar(
            z2d[:], z2d[:], 5.0, -5.0, mybir.AluOpType.min, mybir.AluOpType.max
        )
        z2dh = zpool.tile([P, M], F16)
        nc.vector.tensor_copy(z2dh[:], z2d[:])

        # --- flatten slot-major via SBUF->SBUF DMA on the scalar HWDGE ring
        zrow = zpool.tile([1, N], F16)
        nc.scalar.dma_start(zrow[:].rearrange("a (p m) -> a p m", p=P), z2dh[:])
        ph = php.tile([P, N], F32)
        half = (N // 2 + 127) // 128 * 128  # multiple of 128, <= 512
        assert half <= 512
        zb_bounds = [(0, half), (half, N)]
        for lo, hi in zb_bounds:
            nc.tensor.matmul(
                ph[:, lo:hi],
                ones_row[:],
                zrow[:, lo:hi],
                start=True,
                stop=True,
            )

        # --- hidden chunks: h[c2][p, i] = gelu(z_i * w1[c2*128+p] + b1[..])
        # one chunk per zb half, c2-interleaved (4 ACTs total): amortizes
        # the ~173ns ACT fixed cost while letting the first 4 pairs' GEMMs
        # start after the second ACT
        hid = []
        for c2 in range(2):
            h = hpool.tile([P, N], F16, tag=f"h{c2}", name=f"h{c2}")
            hid.append(h)
        for lo, hi in zb_bounds:
            for c2 in range(2):
                nc.scalar.activation(
                    hid[c2][:, lo:hi],
                    ph[:, lo:hi],
                    mybir.ActivationFunctionType.Gelu,
                    scale=t_vmsb[:, 3 * M + 2 + c2 : 3 * M + 3 + c2],
                    bias=t_vmsb[:, 3 * M + c2 : 3 * M + 1 + c2],
                )

        # --- segment pair GEMMs, column-group packed; drains (DVE casts
        # fp32->fp16) land in one wide SBUF tile so the outputs go out in
        # two wide DMAs instead of seven narrow ones
        osbw = opool.tile([P, PAIRS * E], F16)
        YSPLIT = 4  # pairs [0:4) in the first output DMA, [4:PAIRS) second
        for i in range(PAIRS):
            if i >= len(pos):
                po = pp.tile([P, E], F32, tag="po")
                pos.append(po)
                bias_mms.append(
                    nc.tensor.matmul(
                        po[:],
                        msk,
                        t_bgm[:, i * E : (i + 1) * E],
                        start=True,
                        stop=False,
                        skip_group_check=True,
                    )
                )
            wt = wts[i]
            po = pos[i]
            segs = [2 * i] + ([2 * i + 1] if (2 * i + 1) < S else [])
            last_mm = None
            for c2 in range(2):
                for j, s in enumerate(segs):
                    colbase = 64 * j
                    last_mm = nc.tensor.matmul(
                        po[colbase : colbase + CAP, :],
                        hid[c2][:, s * CAP : (s + 1) * CAP],
                        wt[:, (2 * j + c2) * E : (2 * j + c2 + 1) * E],
                        start=False,
                        stop=(c2 == 1),
                        tile_position=(0, colbase),
                        skip_group_check=True,
                    )
            cp = nc.vector.tensor_copy(osbw[:, i * E : (i + 1) * E], po[:])
            # copy reads the whole tile; deps already cover all matmuls,
            # but order explicitly after the final matmul for bank safety
            add_dep_helper(cp.ins, last_mm.ins, True, "psum drain order")
            if i == YSPLIT - 1:
                nc.scalar.dma_start(
                    y[:, 0 : YSPLIT * E], osbw[:, 0 : YSPLIT * E]
                )
            elif i == PAIRS - 1:
                nc.scalar.dma_start(
                    y[:, YSPLIT * E :], osbw[:, YSPLIT * E :]
                )
    nc.compile()
    return nc


def kernel(values, means, stds, head_idx, w1, b1, W_heads, b_heads):
    global LAST_RESULT
    values = np.ascontiguousarray(values, dtype=np.float32)
    means = np.ascontiguousarray(means, dtype=np.float32)
    stds = np.ascontiguousarray(stds, dtype=np.float32)
    head_idx = np.ascontiguousarray(head_idx, dtype=np.int32)
    w1 = np.ascontiguousarray(w1, dtype=np.float32)
    b1 = np.ascontiguousarray(b1, dtype=np.float32)
    W_heads = np.ascontiguousarray(W_heads, dtype=np.float32)
    b_heads = np.ascontiguousarray(b_heads, dtype=np.float32)
    nb = values.shape[0]

    # ---- host routing: group sample indices by head, chunk to <=64 ----
    order = np.argsort(head_idx, kind="stable")
    counts = np.bincount(head_idx, minlength=K)
    bounds = np.concatenate([[0], np.cumsum(counts)])
    segments = []  # (head, idx_array)
    for k in range(K):
        idx = order[bounds[k] : bounds[k + 1]]
        for lo in range(0, len(idx), CAP):
            segments.append((k, idx[lo : lo + CAP]))
    S = -(-len(segments) // NCORES)
    while len(segments) < S * NCORES:
        segments.append((0, np.empty(0, dtype=np.int64)))
    MCOLS = -(-(S * CAP) // P)
    N = P * MCOLS
    PAIRS = (S + 1) // 2
    NBIAS = PAIRS * E

    key = (S, MCOLS)
    if key not in _build_cache:
        _build_cache[key] = _build(S, MCOLS)
    nc = _build_cache[key]

    b1col = b1.reshape(2, P).T  # (128, 2)
    w1col = w1.reshape(2, P).T  # (128, 2)
    # (K, 128, 2, E): [k, p, c, e] = W_heads[k, c*128+p, e]
    W_chunked = np.ascontiguousarray(
        W_heads.reshape(K, 2, P, E).transpose(0, 2, 1, 3).astype(np.float16)
    )

    in_maps = []
    core_segs = []
    for c in range(NCORES):
        segs = segments[c * S : (c + 1) * S]
        core_segs.append(segs)
        v_slot = np.zeros(N, np.float32)
        m_slot = np.zeros(N, np.float32)
        s_slot = np.ones(N, np.float32)
        for si, (k, idx) in enumerate(segs):
            n = len(idx)
            sl = slice(si * CAP, si * CAP + n)
            v_slot[sl] = values[idx]
            m_slot[sl] = means[idx]
            s_slot[sl] = stds[idx]
        vmsb = np.empty((P, 3 * MCOLS + 4), np.float32)
        vmsb[:, 0:MCOLS] = v_slot.reshape(P, MCOLS)
        vmsb[:, MCOLS : 2 * MCOLS] = m_slot.reshape(P, MCOLS)
        vmsb[:, 2 * MCOLS : 3 * MCOLS] = s_slot.reshape(P, MCOLS)
        vmsb[:, 3 * MCOLS : 3 * MCOLS + 2] = b1col
        vmsb[:, 3 * MCOLS + 2 : 3 * MCOLS + 4] = w1col
        heads = np.array([k for k, _ in segs], np.int64)
        bgm = np.zeros((2, NBIAS + P), np.float16)
        bg = b_heads[heads].astype(np.float16)  # (S, E)
        bgm[0, : (len(segs) + 1) // 2 * E] = bg[0::2].reshape(-1)
        bgm[1, : len(segs) // 2 * E] = bg[1::2].reshape(-1)
        bgm[0, NBIAS : NBIAS + CAP] = 1.0
        bgm[1, NBIAS + CAP : NBIAS + P] = 1.0
        # (128, S*2*E) segment-major, per-partition contiguous
        wgc = np.ascontiguousarray(
            W_chunked[heads].transpose(1, 0, 2, 3).reshape(P, S * 2 * E)
        )
        in_maps.append({"vmsb": vmsb, "bgm": bgm, "wg": wgc})

    res = run_bass_kernel_spmd(nc, in_maps, list(range(NCORES)), trace=TRACE)
    LAST_RESULT = res

    out = np.empty((nb, E), np.float32)
    for c in range(NCORES):
        yc = np.asarray(res.results[c]["y"], dtype=np.float32)  # (128, PAIRS*E)
        for si, (k, idx) in enumerate(core_segs[c]):
            n = len(idx)
            if n:
                r0 = CAP * (si % 2)
                e0 = (si // 2) * E
                out[idx] = yc[r0 : r0 + n, e0 : e0 + E]
    return out
